# revision 1
# baseline (speedup 1.0000x reference)
"""Trainium2 Bass kernel for nn_Block_343597384085 — v3.

Sharding: 8 cores = (batch, seq-half); each core: T=1024 tokens x D=1024,
feature-major SBUF layout [feature-block(128 part), tokens(free)].

Key design vs the v1 baseline:
- x arrives pre-transposed (feature-major, bf16) from the host: no PE
  transposes, no PSUM->SBUF copies.
- alpha and ctx projections run as fp8-e4m3 DoubleRow matmuls (2 k-blocks
  per pass); beta/gate/fin stay bf16 (1 cyc/row, fp22 internal).
  Empirical max-err budget (CPU sim): alpha fp8 ~7e-3/layer, ctx ~2e-3,
  combined plan ~1.4e-2 < 2e-2 gate.
- i=3 ctx projection is dead code (ctx never read after loop) — dropped.
- Elementwise/storage in bf16 (DVE 2x modes); alphas + scan state f32.
- Scan: pass1 in place (h_local, carry column extracted); the
  cumprod(alpha) runs in place on the dead f32 alphas tile DURING the
  pair AllGather (no dependency on the carry), then the correction is
  one per-partition scale + add: fetched = h_local + cumprod*carry.
- ctx/out/v stay resident in SBUF (no DRAM round-trips); og spills.
- ACT ops grouped by table set (rsqrt -> sigmoid(+square) -> sqrt ->
  silu, with Copy/Square free in every set) to kill table-load thrash.
- rms: squares on Pool (bf16), ones-matmul reduce + PE broadcast,
  ACT Sqrt + DVE reciprocal.
"""
import numpy as np
import ml_dtypes

import concourse.bass as bass
import concourse.bacc as bacc
import concourse.mybir as mybir
import concourse.tile as tile
from concourse import bass_utils

B, S, D, N, K = 4, 2048, 1024, 4, 4
EPS = 1e-6
P = 128
NB = D // P             # 8 feature blocks
NP = NB // 2            # 4 fp8 k-pair blocks
T = S // 2              # tokens per core
SUB = 512
NS = T // SUB
F32 = mybir.dt.float32
F32R = mybir.dt.float32r
BF16 = mybir.dt.bfloat16
FP8 = mybir.dt.float8e4
OP = mybir.AluOpType
AF = mybir.ActivationFunctionType
DR = mybir.MatmulPerfMode.DoubleRow

# fp8 scale folding: logits = (SA*cn)@(SW*W) / (SA*SW)
SA_CN = 8.0        # cn ~ N(0,1)
SW_A = 256.0       # alpha_w rms 1/32 -> 8
SA_F = 4.0         # fetched ~ O(1)
SW_C = 8192.0      # ctx_w rms 2e-2/32 -> ~5
SQ_SCALE = 1.0

_CACHE = {}


def _build(reps=1, no_cc=False):
    nc = bacc.Bacc("TRN2", target_bir_lowering=False, debug=False, num_devices=8)

    xt_d = nc.dram_tensor("xt", [D, T + K - 1], BF16, kind="ExternalInput")
    mask_d = nc.dram_tensor("mask", [P, 1], F32, kind="ExternalInput")
    cwp_d = nc.dram_tensor("cwp", [P, NB * K], F32, kind="ExternalInput")
    cbp_d = nc.dram_tensor("cbp", [P, NB], F32, kind="ExternalInput")
    gbp_d = nc.dram_tensor("gbp", [P, NB], F32, kind="ExternalInput")
    rwp_d = nc.dram_tensor("rwp", [P, N * NB], F32, kind="ExternalInput")
    abp_d = nc.dram_tensor("abp", [P, N * NB], F32, kind="ExternalInput")
    bbp_d = nc.dram_tensor("bbp", [P, N * NB], F32, kind="ExternalInput")
    ctbp_d = nc.dram_tensor("ctbp", [P, N * NB], F32, kind="ExternalInput")
    frwp_d = nc.dram_tensor("frwp", [P, NB], F32, kind="ExternalInput")
    fbp_d = nc.dram_tensor("fbp", [P, NB], F32, kind="ExternalInput")
    gw_d = nc.dram_tensor("gate_w", [D, D], BF16, kind="ExternalInput")
    bw_d = nc.dram_tensor("beta_w", [N, D, D], BF16, kind="ExternalInput")
    fw_d = nc.dram_tensor("fin_w", [D, D], BF16, kind="ExternalInput")
    # fp8 pair-packed: [i, kp, p, j, m] = W[i, (2kp+j)*128+p, m] scaled
    aw8_d = nc.dram_tensor("aw8", [N, NP, P, 2 * D], FP8, kind="ExternalInput")
    cw8_d = nc.dram_tensor("cw8", [N - 1, NP, P, 2 * D], FP8, kind="ExternalInput")
    y_d = nc.dram_tensor("y", [D, T], BF16, kind="ExternalOutput")

    with tile.TileContext(nc) as tc:
        _emit(nc, tc, locals(), reps=reps, no_cc=no_cc)
    nc.compile()
    return nc


def _emit(nc, tc, t, reps=1, no_cc=False):
    xt_d = t["xt_d"]; mask_d = t["mask_d"]; cwp_d = t["cwp_d"]
    cbp_d = t["cbp_d"]; gbp_d = t["gbp_d"]; rwp_d = t["rwp_d"]
    abp_d = t["abp_d"]; bbp_d = t["bbp_d"]; ctbp_d = t["ctbp_d"]
    frwp_d = t["frwp_d"]; fbp_d = t["fbp_d"]
    gw_d = t["gw_d"]; bw_d = t["bw_d"]; fw_d = t["fw_d"]
    aw8_d = t["aw8_d"]; cw8_d = t["cw8_d"]; y_d = t["y_d"]

    import contextlib
    with contextlib.ExitStack() as est:
        aux = est.enter_context(tc.tile_pool(name="aux", bufs=1))
        state = est.enter_context(tc.tile_pool(name="state", bufs=1))
        wp = est.enter_context(tc.tile_pool(name="wp", bufs=2))
        w8p = est.enter_context(tc.tile_pool(name="w8p", bufs=1))
        cnp = est.enter_context(tc.tile_pool(name="cnp", bufs=1))
        q8p = est.enter_context(tc.tile_pool(name="q8p", bufs=1))
        alp = est.enter_context(tc.tile_pool(name="alp", bufs=1))
        xtp = est.enter_context(tc.tile_pool(name="xtp", bufs=1))
        sip = est.enter_context(tc.tile_pool(name="sip", bufs=1))
        scr = est.enter_context(tc.tile_pool(name="scr", bufs=2))
        tmp = est.enter_context(tc.tile_pool(name="tmp", bufs=2))
        mmp = est.enter_context(tc.tile_pool(name="mmp", bufs=3, space="PSUM"))
        ssp = est.enter_context(tc.tile_pool(name="ssp", bufs=1, space="PSUM"))
        bcp = est.enter_context(tc.tile_pool(name="bcp", bufs=1, space="PSUM"))
        dram = est.enter_context(tc.tile_pool(name="dram", bufs=1, space="DRAM"))

        def aux_load(name, dram_t, shape, dt=F32):
            tl = aux.tile(shape, dt, name=name)
            nc.sync.dma_start(tl[:], dram_t[:])
            return tl

        mask = aux_load("mask", mask_d, [P, 1])
        cwp = aux_load("cwp", cwp_d, [P, NB * K])
        cbp = aux_load("cbp", cbp_d, [P, NB])
        gbp = aux_load("gbp", gbp_d, [P, NB])
        rwp = aux_load("rwp", rwp_d, [P, N * NB])
        abp = aux_load("abp", abp_d, [P, N * NB])
        bbp = aux_load("bbp", bbp_d, [P, N * NB])
        ctbp = aux_load("ctbp", ctbp_d, [P, N * NB])
        frwp = aux_load("frwp", frwp_d, [P, NB])
        fbp = aux_load("fbp", fbp_d, [P, NB])
        ones_f = aux.tile([P, 1], F32)
        nc.vector.memset(ones_f[:], 1.0)
        eps_t = aux.tile([P, 1], F32)
        nc.vector.memset(eps_t[:], EPS)
        # bf16 ones for the rms reduce / broadcast matmuls
        ones_b = aux.tile([P, 1], BF16)
        nc.vector.tensor_copy(ones_b[:], ones_f[:])
        ones1_b = aux.tile([1, P], BF16)
        o1f = aux.tile([1, P], F32)
        nc.vector.memset(o1f[:], 1.0)
        nc.vector.tensor_copy(ones1_b[:], o1f[:])

        # persistent state (bf16): ctx, out
        ctxb = [state.tile([P, T], BF16, name=f"ctx{nb}") for nb in range(NB)]
        outb = [state.tile([P, T], BF16, name=f"out{nb}") for nb in range(NB)]

        og_s = [dram.tile([P, T], BF16, name=f"og_s{nb}") for nb in range(NB)]
        v_s = [dram.tile([P, T], BF16, name=f"v_s{nb}") for nb in range(NB)]

        def rms_inv(src, sl, tag):
            """[P, SUB] f32 tile of 1/sqrt(mean_d src^2 + eps).
            squares on Pool (bf16 out), ones-matmul reduce (bf16),
            PE broadcast, ACT Rsqrt."""
            ssps = ssp.tile([1, SUB], F32, tag="ss", name=f"ss{tag}")
            for nb in range(NB):
                sq = scr.tile([P, SUB], BF16, tag="sq", name=f"sq{tag}_{nb}")
                nc.gpsimd.tensor_tensor(sq[:], src[nb][:, sl], src[nb][:, sl],
                                        OP.mult)
                nc.tensor.matmul(ssps[:], ones_b[:], sq[:],
                                 start=(nb == 0), stop=(nb == NB - 1))
            ssr = scr.tile([1, SUB], BF16, tag="ssr", name=f"ssr{tag}")
            nc.scalar.copy(ssr[:], ssps[:])
            bc = bcp.tile([P, SUB], F32, tag="bc", name=f"bc{tag}")
            nc.tensor.matmul(bc[:], ones1_b[:], ssr[:], start=True, stop=True)
            sd = tmp.tile([P, SUB], F32, tag="sd", name=f"sd{tag}", bufs=1)
            nc.scalar.activation(sd[:], bc[:], AF.Sqrt, bias=eps_t[:, 0:1],
                                 scale=1.0 / D)
            nc.vector.reciprocal(sd[:], sd[:])
            return sd

        def one_pass(rep):
            r = f"r{rep}"
            # ---- phase 0: x load (feature-major bf16), conv, gate ----
            xT = []
            for nb in range(NB):
                xt = xtp.tile([P, T + K - 1], BF16, tag=f"xt{nb}",
                              name=f"{r}_xT{nb}")
                nc.sync.dma_start(xt[:], xt_d[nb * P:(nb + 1) * P, :])
                xT.append(xt)

            # conv: 4 shifted per-feature MACs on DVE (bf16)
            for nb in range(NB):
                cacc = scr.tile([P, T], BF16, tag="cacc", name=f"{r}_cacc{nb}")
                nc.vector.tensor_scalar(
                    cacc[:], xT[nb][:, 0:T], cwp[:, nb * K:nb * K + 1], None,
                    OP.mult)
                for k in range(1, K):
                    nc.vector.scalar_tensor_tensor(
                        cacc[:], xT[nb][:, k:k + T],
                        cwp[:, nb * K + k:nb * K + k + 1], cacc[:],
                        OP.mult, OP.add)
                nc.scalar.activation(cacc[:], cacc[:], AF.Silu,
                                     bias=cbp[:, nb:nb + 1])
                nc.sync.dma_start(v_s[nb][:], cacc[:])
                nc.vector.tensor_copy(ctxb[nb][:], cacc[:])
                nc.vector.tensor_copy(outb[nb][:], cacc[:])

            # gate: bf16 matmuls, silu -> og -> DRAM spill
            gw = []
            for k in range(NB):
                gwk = wp.tile([P, D], BF16, tag=f"w{k}", name=f"{r}_gw{k}",
                              bufs=2 if k < 2 else 1)
                nc.sync.dma_start(gwk[:], gw_d[k * P:(k + 1) * P, :])
                gw.append(gwk)
            for m in range(NB):
                ps = mmp.tile([P, 2 * SUB], F32, tag="mm", name=f"{r}_psg{m}")
                for k in range(NB):
                    for s in range(NS):
                        nc.tensor.matmul(
                            ps[:, s * SUB:(s + 1) * SUB],
                            gw[k][:, m * P:(m + 1) * P],
                            xT[k][:, K - 1 + s * SUB:K - 1 + (s + 1) * SUB],
                            start=(k == 0), stop=(k == NB - 1),
                            skip_group_check=True)
                ogt = tmp.tile([P, 2 * SUB], BF16, tag="ogt", name=f"{r}_og{m}", bufs=1)
                nc.scalar.activation(ogt[:], ps[:], AF.Silu,
                                     bias=gbp[:, m:m + 1])
                nc.sync.dma_start(og_s[m][:], ogt[:])

            # ---- iterations ----
            for i in range(N):
                # rms + cn (bf16) + cn8 (fp8 pairs)
                cn = [cnp.tile([P, T], BF16, tag=f"cn{nb}", name=f"{r}_cn{i}_{nb}")
                      for nb in range(NB)]
                cn8 = [q8p.tile([P, 2 * T], FP8, tag=f"q8{kp}",
                                name=f"{r}_cn8_{i}_{kp}") for kp in range(NP)]
                for s in range(NS):
                    sl = slice(s * SUB, (s + 1) * SUB)
                    inv = rms_inv(ctxb, sl, f"{r}i{i}s{s}")
                    for nb in range(NB):
                        nc.vector.scalar_tensor_tensor(
                            cn[nb][:, sl], ctxb[nb][:, sl],
                            rwp[:, i * NB + nb:i * NB + nb + 1], inv[:],
                            OP.mult, OP.mult)
                for nb in range(NB):
                    nc.scalar.activation(
                        cn8[nb // 2][:, (nb % 2) * T:(nb % 2 + 1) * T],
                        cn[nb][:], AF.Copy, scale=SA_CN)

                # alpha: fp8 DoubleRow matmuls -> sigmoid -> alphas f32
                wa = []
                for kp in range(NP):
                    wak = w8p.tile([P, 2 * D], FP8, tag=f"a8{kp}",
                                   name=f"{r}_wa{i}_{kp}", bufs=2)
                    nc.sync.dma_start(wak[:], aw8_d[i, kp])
                    wa.append(wak)
                alphas = [alp.tile([P, T], F32, tag=f"al{nb}",
                                   name=f"{r}_alphas{i}_{nb}")
                          for nb in range(NB)]
                for m in range(NB):
                    ps = mmp.tile([P, 2 * SUB], F32, tag="mm",
                                  name=f"{r}_psa{i}_{m}")
                    for kp in range(NP):
                        wap = wa[kp][:].rearrange(
                            "p (j m) -> p j m", j=2)[:, :, m * P:(m + 1) * P]
                        for s in range(NS):
                            sl = slice(s * SUB, (s + 1) * SUB)
                            cnp8 = cn8[kp][:].rearrange(
                                "p (j t) -> p j t", j=2)[:, :, sl]
                            nc.tensor.matmul(
                                ps[:, sl], wap, cnp8, perf_mode=DR,
                                start=(kp == 0), stop=(kp == NP - 1),
                                skip_group_check=True)
                    nc.scalar.activation(
                        alphas[m][:], ps[:], AF.Sigmoid,
                        bias=abp[:, i * NB + m:i * NB + m + 1],
                        scale=1.0 / (SA_CN * SW_A))

                # ws = sqrt(1 - alpha^2) (bf16) — square (free in sigmoid
                # set) then sqrt-set visit; ws reuses dead xT buffers
                wst = [xtp.tile([P, T + K - 1], BF16, tag=f"xt{nb}",
                                name=f"{r}_ws{i}_{nb}") for nb in range(NB)]
                asql = []
                for m in range(NB):
                    asq = tmp.tile([P, T], F32, tag="asq",
                                   name=f"{r}_asq{i}_{m}", bufs=1)
                    nc.scalar.activation(asq[:], alphas[m][:], AF.Square)
                    asql.append((m, asq))
                for m, asq in asql:
                    nc.scalar.activation(wst[m][:, 0:T], asq[:],
                                         AF.Sqrt, bias=ones_f[:, 0:1],
                                         scale=-1.0)

                # beta (bf16 matmul) -> silu -> sin = (beta*ws)*v
                wb = []
                for k in range(NB):
                    wbk = wp.tile([P, D], BF16, tag=f"w{k}", name=f"{r}_wb{i}_{k}",
                                  bufs=2 if k < 2 else 1)
                    nc.sync.dma_start(wbk[:], bw_d[i, k * P:(k + 1) * P, :])
                    wb.append(wbk)
                sin = [sip.tile([P, T], BF16, tag=f"sin{nb}",
                                name=f"{r}_sin{i}_{nb}") for nb in range(NB)]
                carries = scr.tile([P, NB], F32, tag="carr", name=f"{r}_carries{i}")
                for m in range(NB):
                    vw = scr.tile([P, T], BF16, tag="vw", name=f"{r}_vw{i}_{m}")
                    nc.sync.dma_start(vw[:], v_s[m][:])
                    ps = mmp.tile([P, 2 * SUB], F32, tag="mm",
                                  name=f"{r}_psb{i}_{m}")
                    for k in range(NB):
                        for s in range(NS):
                            sl = slice(s * SUB, (s + 1) * SUB)
                            nc.tensor.matmul(ps[:, sl],
                                             wb[k][:, m * P:(m + 1) * P],
                                             cn[k][:, sl],
                                             start=(k == 0), stop=(k == NB - 1),
                                             skip_group_check=True)
                    nc.scalar.activation(sin[m][:], ps[:], AF.Silu,
                                         bias=bbp[:, i * NB + m:i * NB + m + 1])
                    nc.vector.tensor_tensor(sin[m][:], sin[m][:],
                                            wst[m][:, 0:T], OP.mult)
                    nc.gpsimd.tensor_tensor(sin[m][:], sin[m][:], vw[:],
                                            OP.mult)
                # scan pass 1 in place (h_local); cumprod(alpha) in place
                # on the now-dead alphas (f32, v1-proven out=data0=data1
                # pattern) — independent of the carry exchange, so it fills
                # the AllGather latency window
                for m in range(NB):
                    nc.vector.tensor_tensor_scan(sin[m][:], alphas[m][:],
                                                 sin[m][:],
                                                 0.0, OP.mult, OP.add)
                    nc.vector.tensor_copy(carries[:, m:m + 1],
                                          sin[m][:, T - 1:T])
                for m in range(NB):
                    nc.vector.tensor_tensor_scan(alphas[m][:], alphas[m][:],
                                                 alphas[m][:],
                                                 1.0, OP.mult, OP.bypass)

                # carry exchange: pair AllGather; ceff = mask * partner carry
                cin = dram.tile([D], F32, name=f"{r}_cin{i}")
                cout = dram.tile([2, D], F32, name=f"{r}_cout{i}")
                nc.sync.dma_start(cin[:].rearrange("(nb p) -> p nb", p=P),
                                  carries[:])
                if no_cc:
                    nc.sync.dma_start(cout[0:1, :],
                                      cin[:].rearrange("(a b) -> a b", a=1))
                else:
                    nc.gpsimd.collective_compute(
                        "AllGather", OP.bypass,
                        replica_groups=[[0, 1], [2, 3], [4, 5], [6, 7]],
                        ins=[cin.opt()], outs=[cout.opt()])
                gsb = scr.tile([P, NB], F32, tag="gsb", name=f"{r}_gsb{i}")
                nc.sync.dma_start(
                    gsb[:], cout[0:1, :].rearrange("a (nb p) -> (a p) nb", p=P))
                ceff = scr.tile([P, NB], F32, tag="ceff", name=f"{r}_ceff{i}")
                nc.vector.tensor_scalar(ceff[:], gsb[:], mask[:, 0:1], None,
                                        OP.mult)

                # scan pass 2 (init = ceff) in place -> fetched; out += fetched
                f8 = [q8p.tile([P, 2 * T], FP8, tag=f"q8{kp}",
                               name=f"{r}_f8_{i}_{kp}") for kp in range(NP)] \
                    if i < N - 1 else None
                for m in range(NB):
                    nc.vector.tensor_scalar(alphas[m][:], alphas[m][:],
                                            ceff[:, m:m + 1], None, OP.mult)
                    nc.vector.tensor_tensor(sin[m][:], sin[m][:],
                                            alphas[m][:], OP.add)
                    nc.gpsimd.tensor_tensor(outb[m][:], outb[m][:], sin[m][:],
                                            OP.add)
                    if i < N - 1:
                        nc.scalar.activation(
                            f8[m // 2][:, (m % 2) * T:(m % 2 + 1) * T],
                            sin[m][:], AF.Copy, scale=SA_F)

                # ctx += silu(fetched8 @ ctx_w8) (skip dead i=3)
                if i < N - 1:
                    wc = []
                    for kp in range(NP):
                        wck = w8p.tile([P, 2 * D], FP8, tag=f"c8{kp}",
                                       name=f"{r}_wc{i}_{kp}", bufs=2)
                        nc.sync.dma_start(wck[:], cw8_d[i, kp])
                        wc.append(wck)
                    for m in range(NB):
                        ps = mmp.tile([P, 2 * SUB], F32, tag="mm",
                                      name=f"{r}_psc{i}_{m}")
                        for kp in range(NP):
                            wcp = wc[kp][:].rearrange(
                                "p (j m) -> p j m",
                                j=2)[:, :, m * P:(m + 1) * P]
                            for s in range(NS):
                                sl = slice(s * SUB, (s + 1) * SUB)
                                f8p = f8[kp][:].rearrange(
                                    "p (j t) -> p j t", j=2)[:, :, sl]
                                nc.tensor.matmul(
                                    ps[:, sl], wcp, f8p, perf_mode=DR,
                                    start=(kp == 0), stop=(kp == NP - 1),
                                    skip_group_check=True)
                        cu = tmp.tile([P, 2 * SUB], BF16, tag="cu",
                                      name=f"{r}_cu{i}_{m}")
                        nc.scalar.activation(
                            cu[:], ps[:], AF.Silu,
                            bias=ctbp[:, i * NB + m:i * NB + m + 1],
                            scale=1.0 / (SA_F * SW_C))
                        nc.gpsimd.tensor_tensor(ctxb[m][:],
                                                ctxb[m][:], cu[:],
                                                OP.add)

            # ---- final: y = silu(rmsnorm(out*og)*fin_rms_w @ fin_w + b) ----
            po = [cnp.tile([P, T], BF16, tag=f"cn{nb}", name=f"{r}_po{nb}")
                  for nb in range(NB)]
            for nb in range(NB):
                ogl = scr.tile([P, T], BF16, tag="ogl", name=f"{r}_ogl{nb}")
                nc.sync.dma_start(ogl[:], og_s[nb][:])
                nc.vector.tensor_tensor(po[nb][:], outb[nb][:], ogl[:], OP.mult)
            fo = [sip.tile([P, T], BF16, tag=f"sin{nb}", name=f"{r}_fo{nb}")
                  for nb in range(NB)]
            for s in range(NS):
                sl = slice(s * SUB, (s + 1) * SUB)
                inv = rms_inv(po, sl, f"{r}fin{s}")
                for nb in range(NB):
                    nc.vector.scalar_tensor_tensor(
                        fo[nb][:, sl], po[nb][:, sl], frwp[:, nb:nb + 1],
                        inv[:], OP.mult, OP.mult)
            fw = []
            for k in range(NB):
                fwk = wp.tile([P, D], BF16, tag=f"w{k}", name=f"{r}_fw{k}",
                              bufs=2 if k < 2 else 1)
                nc.sync.dma_start(fwk[:], fw_d[k * P:(k + 1) * P, :])
                fw.append(fwk)
            for m in range(NB):
                ps = mmp.tile([P, 2 * SUB], F32, tag="mm", name=f"{r}_psf{m}")
                for k in range(NB):
                    for s in range(NS):
                        sl = slice(s * SUB, (s + 1) * SUB)
                        nc.tensor.matmul(ps[:, sl],
                                         fw[k][:, m * P:(m + 1) * P],
                                         fo[k][:, sl],
                                         start=(k == 0), stop=(k == NB - 1),
                                         skip_group_check=True)
                yt = tmp.tile([P, 2 * SUB], BF16, tag="yt", name=f"{r}_yt{m}", bufs=1)
                nc.scalar.activation(yt[:], ps[:], AF.Silu,
                                     bias=fbp[:, m:m + 1])
                nc.sync.dma_start(y_d[m * P:(m + 1) * P, :], yt[:])

        for rep in range(reps):
            one_pass(rep)


def _q8pack(w, scale):
    """[D, D] f32 -> [NP, P, 2*D] e4m3 pair-packed, scaled."""
    x = np.clip(np.asarray(w, np.float32) * scale, -240.0, 240.0)
    q = x.astype(ml_dtypes.float8_e4m3fn)
    # [kp, p, j, m] = q[(2kp+j)*128 + p, m]
    q = q.reshape(NP, 2, P, D).transpose(0, 2, 1, 3).reshape(NP, P, 2 * D)
    return np.ascontiguousarray(q)


def _prep_in_maps(inputs):
    f32 = lambda k: np.asarray(inputs[k], np.float32)
    bf = ml_dtypes.bfloat16
    x = f32("x"); conv_w = f32("conv_w"); conv_b = f32("conv_b")
    gate_w = f32("gate_w"); gate_b = f32("gate_b"); rms_w = f32("rms_w")
    alpha_w = f32("alpha_w"); alpha_b = f32("alpha_b")
    beta_w = f32("beta_w"); beta_b = f32("beta_b")
    ctx_w = f32("ctx_w"); ctx_b = f32("ctx_b")
    fin_rms_w = f32("fin_rms_w"); fin_w = f32("fin_w"); fin_b = f32("fin_b")

    def pack1(a):
        return np.ascontiguousarray(a.reshape(NB, P).T)

    def packN(a):
        return np.ascontiguousarray(
            a.reshape(N, NB, P).transpose(2, 0, 1).reshape(P, N * NB))

    cwp = np.ascontiguousarray(
        conv_w.T.reshape(NB, P, K).transpose(1, 0, 2).reshape(P, NB * K))
    aw8 = np.stack([_q8pack(alpha_w[i], SW_A) for i in range(N)])
    cw8 = np.stack([_q8pack(ctx_w[i], SW_C) for i in range(N - 1)])
    shared = dict(
        cwp=cwp, cbp=pack1(conv_b), gbp=pack1(gate_b),
        rwp=packN(rms_w), abp=packN(alpha_b), bbp=packN(beta_b),
        ctbp=packN(ctx_b), frwp=pack1(fin_rms_w), fbp=pack1(fin_b),
        gate_w=np.ascontiguousarray(gate_w.astype(bf)),
        beta_w=np.ascontiguousarray(beta_w.astype(bf)),
        fin_w=np.ascontiguousarray(fin_w.astype(bf)),
        aw8=aw8, cw8=cw8,
    )
    in_maps = []
    for c in range(8):
        b, h = c // 2, c % 2
        t0 = h * T
        m = dict(shared)
        xt = np.zeros((D, T + K - 1), np.float32)
        xt[:, K - 1:] = x[b, t0:t0 + T].T
        if h == 1:
            xt[:, 0:K - 1] = x[b, t0 - (K - 1):t0].T
        m["xt"] = np.ascontiguousarray(xt.astype(bf))
        m["mask"] = np.full((P, 1), float(h), np.float32)
        in_maps.append(m)
    return in_maps


def kernel(**inputs) -> np.ndarray:
    if "nc" not in _CACHE:
        _CACHE["nc"] = _build()
    nc = _CACHE["nc"]
    in_maps = _prep_in_maps(inputs)
    res = bass_utils.run_bass_kernel_spmd(nc, in_maps, core_ids=list(range(8)))
    y = np.empty((B, S, D), np.float32)
    for c in range(8):
        b, h = c // 2, c % 2
        y[b, h * T:(h + 1) * T] = np.asarray(res.results[c]["y"],
                                             np.float32).T
    return y

